# revision 4
# baseline (speedup 1.0000x reference)
"""BandSimVQ Trainium2 kernel (8 NeuronCores, SPMD data-parallel over batch).

Reference computation (per batch b, band k):
    implicit[c,e] = sum_d codebooks[k,c,d] * W[k,d,e]          # [CS, D]
    d2[t,c]      = ||x[b,k,:,t] - implicit[c,:]||^2
    idx[t]       = argmin_c d2[t,c]
    q[e,t]       = implicit[idx[t], e]
    loss         = 1.25 * mean_{b,k,t} min_c d2[t,c]
Outputs: (quantized=[B,K,D,T] f32, indices=[B,K,T] i32, loss scalar f32).

Kernel strategy: core b handles batch b (no collectives).  Per band:
  implicitT[e,c] = matmul(lhsT=W[d,e], rhs=cbT[d,c]) accumulated over d
  score[t,c]     = matmul(lhsT=x[e,t], rhs=implicitT[e,c]) - c2[c]/2
  argmin via vector max8/max_index8, q via gpsimd ap_gather on implicitT,
  loss via activation-accumulated ||x||^2 and the score maxima.
"""

import numpy as np

B, NB, D, T = 8, 6, 512, 768     # batch, bands, feature dim, frames
CS, CD = 2048, 512               # codebook size, codebook dim
NCORES = 8

ETILES = D // 128                # 4  (e = output feature dim)
DTILES = CD // 128               # 4  (d = codebook dim, contraction)
TTILES = T // 128                # 6
CCHUNK = 256                     # c-chunk width for matmul free dim
NCCH = CS // CCHUNK              # 8


def _build_bass():
    import concourse.bass as bass
    import concourse.mybir as mybir
    from concourse import bacc
    from concourse.tile import TileContext

    f32 = mybir.dt.float32
    i32 = mybir.dt.int32
    i16 = mybir.dt.int16
    u32 = mybir.dt.uint32

    nc = bacc.Bacc(None, target_bir_lowering=False, debug=False)

    x_ext = nc.declare_dram_parameter("x", [NB, D, T], f32, isOutput=False)
    cbT_ext = nc.declare_dram_parameter("cbT", [NB, CD, CS], f32, isOutput=False)
    w_ext = nc.declare_dram_parameter("w", [NB, CD, D], f32, isOutput=False)
    outq_ext = nc.declare_dram_parameter("out_q", [NB, D, T], f32, isOutput=True)
    outi_ext = nc.declare_dram_parameter("out_idx", [NB, T], i32, isOutput=True)
    outp_ext = nc.declare_dram_parameter("out_partial", [1, 1], f32, isOutput=True)

    # internal DRAM scratch for the index-layout round trip
    idx_scr = nc.dram_tensor("idx_scr", [NB, T], i16)

    with TileContext(nc) as tc:
        with (
            tc.tile_pool(name="weights", bufs=1) as wpool,
            tc.tile_pool(name="cbt", bufs=1) as cbtpool,
            tc.tile_pool(name="xband", bufs=2) as xpool,
            tc.tile_pool(name="implt", bufs=1) as iplpool,
            tc.tile_pool(name="score", bufs=2) as scpool,
            tc.tile_pool(name="small", bufs=2) as smpool,
            tc.tile_pool(name="qout", bufs=1) as qpool,
            tc.tile_pool(name="psum_i", bufs=2, space="PSUM") as ppool_i,
            tc.tile_pool(name="psum_s", bufs=3, space="PSUM") as ppool_s,
            tc.tile_pool(name="psum_c2", bufs=2, space="PSUM") as ppool_c2,
            tc.tile_pool(name="psum_fin", bufs=1, space="PSUM") as ppool_fin,
        ):
            ones_sb = wpool.tile([128, 1], f32, tag="ones")
            nc.vector.memset(ones_sb[:], 1.0)

            # accumulator columns: 0..NB*ETILES-1 hold sum_t x^2 per (band,etile)
            # cols 24..24+NB*TTILES-1 hold -2*smax sums per (band,ttile)
            NACC = NB * ETILES + NB * TTILES          # 24 + 36 = 60
            acc_all = wpool.tile([128, NACC], f32, tag="acc")
            nc.vector.memset(acc_all[:], 0.0)

            for k in range(NB):
                # ---- load band data ----
                w_sb = [wpool.tile([128, D], f32, tag=f"w{di}", name=f"w_sb{di}") for di in range(DTILES)]
                for di in range(DTILES):
                    nc.sync.dma_start(w_sb[di][:], w_ext[k, 128 * di:128 * (di + 1), :])
                cbT_sb = [cbtpool.tile([128, CS], f32, tag=f"cbt{di}", name=f"cbt_sb{di}") for di in range(DTILES)]
                for di in range(DTILES):
                    nc.sync.dma_start(cbT_sb[di][:], cbT_ext[k, 128 * di:128 * (di + 1), :])
                x_sb = [xpool.tile([128, T], f32, tag=f"x{ei}", name=f"x_sb{ei}") for ei in range(ETILES)]
                for ei in range(ETILES):
                    nc.sync.dma_start(x_sb[ei][:], x_ext[k, 128 * ei:128 * (ei + 1), :])

                # ---- implicitT[e,c] and c2[c] ----
                implT_sb = [iplpool.tile([128, CS], f32, tag=f"ipl{ei}", name=f"implT_sb{ei}") for ei in range(ETILES)]
                c2h_sb = smpool.tile([1, CS], f32, tag="c2h")
                for cj in range(NCCH):
                    csl = bass.ts(cj, CCHUNK)
                    c2_ps = ppool_c2.tile([1, CCHUNK], f32)
                    for ei in range(ETILES):
                        esl = bass.ts(ei, 128)
                        impl_ps = ppool_i.tile([128, CCHUNK], f32)
                        for di in range(DTILES):
                            nc.tensor.matmul(
                                impl_ps[:], w_sb[di][:, esl], cbT_sb[di][:, csl],
                                start=(di == 0), stop=(di == DTILES - 1),
                            )
                        nc.scalar.copy(implT_sb[ei][:, csl], impl_ps[:])
                        sq_sb = smpool.tile([128, CCHUNK], f32, tag="sq")
                        nc.vector.tensor_mul(sq_sb[:], implT_sb[ei][:, csl], implT_sb[ei][:, csl])
                        nc.tensor.matmul(
                            c2_ps[:], ones_sb[:], sq_sb[:],
                            start=(ei == 0), stop=(ei == ETILES - 1),
                        )
                    # c2/2 for the score subtraction
                    nc.scalar.mul(c2h_sb[:, csl], c2_ps[:], 0.5)
                c2rep = cbtpool.tile([128, CS], f32, tag="c2rep")
                nc.gpsimd.partition_broadcast(c2rep[:], c2h_sb[:], channels=128)

                # ---- score, argmax, gather ----
                idx16_sb = smpool.tile([128, TTILES], i16, tag="idx16")
                for tt in range(TTILES):
                    tsl = bass.ts(tt, 128)
                    score_sb = scpool.tile([128, CS], f32, tag="score")
                    for cj in range(NCCH):
                        csl = bass.ts(cj, CCHUNK)
                        sc_ps = ppool_s.tile([128, CCHUNK], f32)
                        for ei in range(ETILES):
                            nc.tensor.matmul(
                                sc_ps[:], x_sb[ei][:, tsl], implT_sb[ei][:, csl],
                                start=(ei == 0), stop=(ei == ETILES - 1),
                            )
                        nc.vector.tensor_sub(score_sb[:, csl], sc_ps[:], c2rep[:, csl])
                    mx8 = smpool.tile([128, 8], f32, tag="mx8")
                    ix8 = smpool.tile([128, 8], u32, tag="ix8")
                    nc.vector.max(mx8[:], score_sb[:])
                    nc.vector.max_index(ix8[:], mx8[:], score_sb[:])
                    # indices out (int32)
                    idx32_sb = smpool.tile([128, 1], i32, tag="idx32")
                    nc.vector.tensor_copy(idx32_sb[:], ix8[:, 0:1])
                    nc.sync.dma_start(outi_ext[k, 128 * tt:128 * (tt + 1)], idx32_sb[:])
                    # int16 copy for the gather index table
                    nc.vector.tensor_copy(idx16_sb[:, tt:tt + 1], ix8[:, 0:1])
                    # -2 * smax into the accumulator
                    nc.vector.tensor_scalar_mul(
                        acc_all[:, NB * ETILES + k * TTILES + tt: NB * ETILES + k * TTILES + tt + 1],
                        mx8[:, 0:1], -2.0,
                    )

                # ---- index layout round trip (t-order, then 16-wrapped) ----
                nc.sync.dma_start(
                    idx_scr[k].rearrange("(c p) -> p c", p=128), idx16_sb[:]
                )
                idxw_sb = smpool.tile([128, T // 16], i16, tag="idxw")
                wr_view = idx_scr[k].rearrange("(s r) -> r s", r=16)
                for g in range(8):
                    nc.sync.dma_start(idxw_sb[16 * g:16 * (g + 1), :], wr_view)

                # ---- q gather + x2 accumulation ----
                for ei in range(ETILES):
                    gq_sb = qpool.tile([128, T], f32, tag=f"gq{ei}", name=f"gq_sb{ei}")
                    nc.gpsimd.ap_gather(
                        gq_sb[:], implT_sb[ei][:], idxw_sb[:],
                        channels=128, num_elems=CS, d=1, num_idxs=T,
                    )
                    nc.sync.dma_start(outq_ext[k, 128 * ei:128 * (ei + 1), :], gq_sb[:])
                    # sum_t x^2 for this (band, etile) via activation accumulate
                    xsq_sb = smpool.tile([128, T], f32, tag="xsq")
                    nc.scalar.activation(
                        xsq_sb[:], x_sb[ei][:],
                        mybir.ActivationFunctionType.Square,
                        accum_out=acc_all[:, k * ETILES + ei: k * ETILES + ei + 1],
                    )

            # ---- final loss partial: sum over accumulator ----
            fin_ps = ppool_fin.tile([1, NACC], f32)
            nc.tensor.matmul(fin_ps[:], ones_sb[:], acc_all[:], start=True, stop=True)
            fin_sb = smpool.tile([1, NACC], f32, tag="fin")
            nc.vector.tensor_copy(fin_sb[:], fin_ps[:])
            part_sb = smpool.tile([1, 1], f32, tag="part")
            nc.vector.tensor_reduce(
                part_sb[:], fin_sb[:], mybir.AxisListType.X, mybir.AluOpType.add
            )
            nc.sync.dma_start(outp_ext[:], part_sb[:])

    nc.finalize()
    return nc


_NC = None


def kernel(x, codebooks, W):
    from concourse.bass_utils import run_bass_kernel_spmd

    global _NC
    if _NC is None:
        _NC = _build_bass()

    x = np.ascontiguousarray(x, dtype=np.float32)
    cbT = np.ascontiguousarray(codebooks.transpose(0, 2, 1), dtype=np.float32)
    w = np.ascontiguousarray(W, dtype=np.float32)

    in_maps = [{"x": x[b], "cbT": cbT, "w": w} for b in range(NCORES)]
    res = run_bass_kernel_spmd(_NC, in_maps, list(range(NCORES)))

    quant = np.stack([res.results[b]["out_q"] for b in range(NCORES)])
    idx = np.stack([res.results[b]["out_idx"] for b in range(NCORES)])
    partial = sum(float(res.results[b]["out_partial"][0, 0]) for b in range(NCORES))
    loss = np.float32(1.25 * partial / (B * NB * T))
    return quant, idx.astype(np.int32), loss


# revision 5
# speedup vs baseline: 1.2828x; 1.2828x over previous
"""BandSimVQ Trainium2 kernel (8 NeuronCores, SPMD data-parallel over batch).

Reference computation (per batch b, band k):
    implicit[c,e] = sum_d codebooks[k,c,d] * W[k,d,e]          # [CS, D]
    d2[t,c]      = ||x[b,k,:,t] - implicit[c,:]||^2
    idx[t]       = argmin_c d2[t,c]
    q[e,t]       = implicit[idx[t], e]
    loss         = 1.25 * mean_{b,k,t} min_c d2[t,c]
Outputs: (quantized=[B,K,D,T] f32, indices=[B,K,T] i32, loss scalar f32).

Kernel strategy: core b handles batch b (no collectives).  Per band:
  implicitT[e,c] = matmul(lhsT=W[d,e], rhs=cbT[d,c]) accumulated over d
  score[t,c]     = matmul(lhsT=x[e,t], rhs=implicitT[e,c]) - c2[c]/2
  argmin via vector max8/max_index8, q via gpsimd ap_gather on implicitT,
  loss via activation-accumulated ||x||^2 and the score maxima.
"""

import numpy as np

B, NB, D, T = 8, 6, 512, 768     # batch, bands, feature dim, frames
CS, CD = 2048, 512               # codebook size, codebook dim
NCORES = 8

ETILES = D // 128                # 4  (e = output feature dim)
DTILES = CD // 128               # 4  (d = codebook dim, contraction)
TTILES = T // 128                # 6
CCHUNK = 256                     # c-chunk width for matmul free dim
NCCH = CS // CCHUNK              # 8


def _build_bass():
    import concourse.bass as bass
    import concourse.mybir as mybir
    from concourse import bacc
    from concourse.tile import TileContext

    f32 = mybir.dt.float32
    i32 = mybir.dt.int32
    i16 = mybir.dt.int16
    u32 = mybir.dt.uint32

    nc = bacc.Bacc(None, target_bir_lowering=False, debug=False)

    x_ext = nc.declare_dram_parameter("x", [NB, D, T], f32, isOutput=False)
    cbT_ext = nc.declare_dram_parameter("cbT", [NB, CD, CS], f32, isOutput=False)
    w_ext = nc.declare_dram_parameter("w", [NB, CD, D], f32, isOutput=False)
    outq_ext = nc.declare_dram_parameter("out_q", [NB, D, T], f32, isOutput=True)
    outi_ext = nc.declare_dram_parameter("out_idx", [NB, T], i32, isOutput=True)
    outp_ext = nc.declare_dram_parameter("out_partial", [1, 1], f32, isOutput=True)

    # internal DRAM scratch for the index-layout round trip
    idx_scr = nc.dram_tensor("idx_scr", [NB, T], i16)

    with TileContext(nc) as tc:
        with (
            tc.tile_pool(name="weights", bufs=1) as wpool,
            tc.tile_pool(name="cbt", bufs=1) as cbtpool,
            tc.tile_pool(name="xband", bufs=2) as xpool,
            tc.tile_pool(name="implt", bufs=2) as iplpool,
            tc.tile_pool(name="score", bufs=2) as scpool,
            tc.tile_pool(name="small", bufs=2) as smpool,
            tc.tile_pool(name="qout", bufs=1) as qpool,
            tc.tile_pool(name="psum_i", bufs=2, space="PSUM") as ppool_i,
            tc.tile_pool(name="psum_s", bufs=3, space="PSUM") as ppool_s,
            tc.tile_pool(name="psum_c2", bufs=2, space="PSUM") as ppool_c2,
            tc.tile_pool(name="psum_fin", bufs=1, space="PSUM") as ppool_fin,
        ):
            ones_sb = wpool.tile([128, 1], f32, tag="ones")
            nc.vector.memset(ones_sb[:], 1.0)
            ones_bf = wpool.tile([128, 1], mybir.dt.bfloat16, tag="onesbf")
            nc.vector.memset(ones_bf[:], 1.0)

            # accumulator columns: 0..NB*ETILES-1 hold sum_t x^2 per (band,etile)
            # cols 24..24+NB*TTILES-1 hold -2*smax sums per (band,ttile)
            NACC = NB * ETILES + NB * TTILES          # 24 + 36 = 60
            acc_all = wpool.tile([128, NACC], f32, tag="acc")
            nc.vector.memset(acc_all[:], 0.0)

            for k in range(NB):
                # ---- load band data ----
                w_sb = [wpool.tile([128, D], f32, tag=f"w{di}", name=f"w_sb{di}") for di in range(DTILES)]
                for di in range(DTILES):
                    nc.sync.dma_start(w_sb[di][:], w_ext[k, 128 * di:128 * (di + 1), :])
                cbT_sb = [cbtpool.tile([128, CS], f32, tag=f"cbt{di}", name=f"cbt_sb{di}") for di in range(DTILES)]
                for di in range(DTILES):
                    nc.sync.dma_start(cbT_sb[di][:], cbT_ext[k, 128 * di:128 * (di + 1), :])
                x_sb = [xpool.tile([128, T], f32, tag=f"x{ei}", name=f"x_sb{ei}") for ei in range(ETILES)]
                for ei in range(ETILES):
                    nc.sync.dma_start(x_sb[ei][:], x_ext[k, 128 * ei:128 * (ei + 1), :])

                # ---- implicitT[e,c] and c2[c] ----
                implT_sb = [iplpool.tile([128, CS], f32, tag=f"ipl{ei}", name=f"implT_sb{ei}") for ei in range(ETILES)]
                c2h_sb = smpool.tile([1, CS], f32, tag="c2h")
                for cj in range(NCCH):
                    csl = bass.ts(cj, CCHUNK)
                    c2_ps = ppool_c2.tile([1, CCHUNK], f32)
                    for ei in range(ETILES):
                        esl = bass.ts(ei, 128)
                        impl_ps = ppool_i.tile([128, CCHUNK], f32)
                        for di in range(DTILES):
                            nc.tensor.matmul(
                                impl_ps[:], w_sb[di][:, esl], cbT_sb[di][:, csl],
                                start=(di == 0), stop=(di == DTILES - 1),
                            )
                        nc.scalar.copy(implT_sb[ei][:, csl], impl_ps[:])
                        sq_sb = smpool.tile([128, CCHUNK], mybir.dt.bfloat16, tag="sq")
                        nc.vector.tensor_mul(sq_sb[:], implT_sb[ei][:, csl], implT_sb[ei][:, csl])
                        nc.tensor.matmul(
                            c2_ps[:], ones_bf[:], sq_sb[:],
                            start=(ei == 0), stop=(ei == ETILES - 1),
                        )
                    # c2/2 for the score subtraction
                    nc.scalar.mul(c2h_sb[:, csl], c2_ps[:], 0.5)
                c2rep = cbtpool.tile([128, CS], f32, tag="c2rep")
                nc.gpsimd.partition_broadcast(c2rep[:], c2h_sb[:], channels=128)

                # ---- score, argmax, gather ----
                idx16_sb = smpool.tile([128, TTILES], i16, tag="idx16")
                for tt in range(TTILES):
                    tsl = bass.ts(tt, 128)
                    score_sb = scpool.tile([128, CS], f32, tag="score")
                    for cj in range(NCCH):
                        csl = bass.ts(cj, CCHUNK)
                        sc_ps = ppool_s.tile([128, CCHUNK], f32)
                        for ei in range(ETILES):
                            nc.tensor.matmul(
                                sc_ps[:], x_sb[ei][:, tsl], implT_sb[ei][:, csl],
                                start=(ei == 0), stop=(ei == ETILES - 1),
                            )
                        nc.vector.tensor_sub(score_sb[:, csl], sc_ps[:], c2rep[:, csl])
                    mx8 = smpool.tile([128, 8], f32, tag="mx8")
                    ix8 = smpool.tile([128, 8], u32, tag="ix8")
                    nc.vector.max(mx8[:], score_sb[:])
                    nc.vector.max_index(ix8[:], mx8[:], score_sb[:])
                    # indices out (int32)
                    idx32_sb = smpool.tile([128, 1], i32, tag="idx32")
                    nc.vector.tensor_copy(idx32_sb[:], ix8[:, 0:1])
                    nc.sync.dma_start(outi_ext[k, 128 * tt:128 * (tt + 1)], idx32_sb[:])
                    # int16 copy for the gather index table
                    nc.vector.tensor_copy(idx16_sb[:, tt:tt + 1], ix8[:, 0:1])
                    # -2 * smax into the accumulator
                    nc.vector.tensor_scalar_mul(
                        acc_all[:, NB * ETILES + k * TTILES + tt: NB * ETILES + k * TTILES + tt + 1],
                        mx8[:, 0:1], -2.0,
                    )

                # ---- index layout round trip (t-order, then 16-wrapped) ----
                nc.sync.dma_start(
                    idx_scr[k].rearrange("(c p) -> p c", p=128), idx16_sb[:]
                )
                idxw_sb = smpool.tile([128, T // 16], i16, tag="idxw")
                wr_view = idx_scr[k].rearrange("(s r) -> r s", r=16)
                for g in range(8):
                    nc.sync.dma_start(idxw_sb[16 * g:16 * (g + 1), :], wr_view)

                # ---- q gather + x2 accumulation ----
                for ei in range(ETILES):
                    gq_sb = qpool.tile([128, T], f32, tag=f"gq{ei}", name=f"gq_sb{ei}")
                    nc.gpsimd.ap_gather(
                        gq_sb[:], implT_sb[ei][:], idxw_sb[:],
                        channels=128, num_elems=CS, d=1, num_idxs=T,
                    )
                    nc.sync.dma_start(outq_ext[k, 128 * ei:128 * (ei + 1), :], gq_sb[:])
                    # sum_t x^2 for this (band, etile) via activation accumulate
                    xsq_sb = smpool.tile([128, T], f32, tag="xsq")
                    nc.scalar.activation(
                        xsq_sb[:], x_sb[ei][:],
                        mybir.ActivationFunctionType.Square,
                        accum_out=acc_all[:, k * ETILES + ei: k * ETILES + ei + 1],
                    )

            # ---- final loss partial: sum over accumulator ----
            fin_ps = ppool_fin.tile([1, NACC], f32)
            nc.tensor.matmul(fin_ps[:], ones_sb[:], acc_all[:], start=True, stop=True)
            fin_sb = smpool.tile([1, NACC], f32, tag="fin")
            nc.vector.tensor_copy(fin_sb[:], fin_ps[:])
            part_sb = smpool.tile([1, 1], f32, tag="part")
            nc.vector.tensor_reduce(
                part_sb[:], fin_sb[:], mybir.AxisListType.X, mybir.AluOpType.add
            )
            nc.sync.dma_start(outp_ext[:], part_sb[:])

    nc.finalize()
    return nc


_NC = None


def kernel(x, codebooks, W):
    from concourse.bass_utils import run_bass_kernel_spmd

    global _NC
    if _NC is None:
        _NC = _build_bass()

    x = np.ascontiguousarray(x, dtype=np.float32)
    cbT = np.ascontiguousarray(codebooks.transpose(0, 2, 1), dtype=np.float32)
    w = np.ascontiguousarray(W, dtype=np.float32)

    in_maps = [{"x": x[b], "cbT": cbT, "w": w} for b in range(NCORES)]
    res = run_bass_kernel_spmd(_NC, in_maps, list(range(NCORES)))

    quant = np.stack([res.results[b]["out_q"] for b in range(NCORES)])
    idx = np.stack([res.results[b]["out_idx"] for b in range(NCORES)])
    partial = sum(float(res.results[b]["out_partial"][0, 0]) for b in range(NCORES))
    loss = np.float32(1.25 * partial / (B * NB * T))
    return quant, idx.astype(np.int32), loss
